# revision 1
# baseline (speedup 1.0000x reference)
"""MultiHeadAttention kernel for Trainium2, 8-core head-parallel.

Problem: S=2048, B=2, D=1024, 16 heads of d=64 (batch_first=False).
Sharding: tensor-parallel over heads — each of the 8 cores computes 2 heads
(a 128-column slice of the output). No collectives: every core gets the full
(bf16, transposed) activations plus its own weight slice, computes its output
slice, and the host concatenates.

Per-core dataflow (layouts chosen so only V needs an on-device transpose,
done on the PE):
  q^T, k^T  [128=2*64 dout, T] = W_slice @ x^T          (PE, bf16, fp32 psum)
  v^T       [128, T] likewise, then PE-transposed to token-major v' [tok, 65]
            per head with a ones column appended (for softmax denominators)
  scores^T  [j, i] = k_h-tile . q_h-tile                (PE, K=64; the two
            heads' matmuls are emitted adjacently at partition bases 0/64 so
            they row-pack and run concurrently in the 128x128 array)
  attn^T    = exp(scores * 1/8)  [no max-subtract: scores ~ N(8, 1.7)]
                                                        (ScalarE, bf16 out)
  pv^T      [65, i] = v'^T . attn^T  — row 64 = softmax denominator,
            accumulated per j-tile right behind the exps (keeps PE dense)
  out^T     [64, i] = pv^T[0:64] * (1/pv^T[64])         (DVE; reciprocal on a
            [128, 8] reshape — single-partition reciprocal is ~6.5us on DVE)
Host gathers out^T [128, B*S] per core -> [S, B, 1024].
"""

import sys

if "/opt/trn_rl_repo" not in sys.path:
    sys.path.insert(0, "/opt/trn_rl_repo")

import numpy as np
import ml_dtypes

import concourse.bass as bass
import concourse.mybir as mybir
import concourse.tile as tile
from concourse import bacc

BF16 = mybir.dt.bfloat16
FP32 = mybir.dt.float32
NP_BF16 = ml_dtypes.bfloat16

D = 1024
NHEAD = 16
DH = 64
NCORES = 8
HPC = NHEAD // NCORES        # heads per core = 2
DC = HPC * DH                # per-core output dims = 128
KT = D // 128                # contraction tiles = 8
SCALE = 1.0 / float(np.sqrt(DH))


def build_program(S: int, B: int):
    """Build the single-core Bass program (identical across the 8 cores)."""
    assert S % 128 == 0
    T = S * B
    JT = S // 128                    # key tiles per (b, h)
    IC = min(1024, S)                # i-chunk (query positions per psum tile)
    assert S % IC == 0
    NIC = S // IC
    NI5 = IC // 512 if IC >= 512 else 1   # 512-wide matmuls per i-chunk
    MMW = IC // NI5                  # matmul free size (<=512)
    TB = 512 if S % 512 == 0 else S  # token block for projections
    TPB = S // TB                    # token blocks per batch
    VSUB = TB // 128                 # 128-token v tiles per block
    JTB = S // 128                   # v tiles per batch

    nc = bacc.Bacc(
        "TRN2", target_bir_lowering=False, debug=False, num_devices=NCORES
    )
    NTILE = T // TB
    # pre-tiled on host: tile (b*TPB+tb) is one contiguous [128, KT, TB] block
    xq = nc.dram_tensor("xq", [NTILE, 128, KT, TB], BF16, kind="ExternalInput")
    xk = nc.dram_tensor("xk", [NTILE, 128, KT, TB], BF16, kind="ExternalInput")
    xv = nc.dram_tensor("xv", [NTILE, 128, KT, TB], BF16, kind="ExternalInput")
    wq = nc.dram_tensor("wq", [D, DC], BF16, kind="ExternalInput")
    wk = nc.dram_tensor("wk", [D, DC], BF16, kind="ExternalInput")
    wv = nc.dram_tensor("wv", [D, DC], BF16, kind="ExternalInput")
    bqkv = nc.dram_tensor("bqkv", [DC, 3], FP32, kind="ExternalInput")
    ident = nc.dram_tensor("ident", [128, 128], BF16, kind="ExternalInput")
    out = nc.dram_tensor("out", [DC, T], FP32, kind="ExternalOutput")


    with tile.TileContext(nc) as tc:
        with (
            tc.tile_pool(name="const", bufs=1) as constp,
            tc.tile_pool(name="xin", bufs=1) as xinp,
            tc.tile_pool(name="qkv", bufs=1) as qkvp,
            tc.tile_pool(name="attn", bufs=3) as attnp,
            tc.tile_pool(name="vstg", bufs=2) as vstgp,
            tc.tile_pool(name="drain", bufs=1) as drainp,
            tc.tile_pool(name="outp", bufs=2) as outp,
            tc.tile_pool(name="ps", bufs=2, space="PSUM") as psp,
            tc.tile_pool(name="pv", bufs=2, space="PSUM") as pvp,
        ):
            wq_t = constp.tile([128, KT, DC], BF16, tag="wq")
            wk_t = constp.tile([128, KT, DC], BF16, tag="wk")
            wv_t = constp.tile([128, KT, DC], BF16, tag="wv")
            nc.sync.dma_start(out=wq_t[:], in_=wq[:, :].rearrange("(kt p) m -> p kt m", p=128))
            nc.sync.dma_start(out=wk_t[:], in_=wk[:, :].rearrange("(kt p) m -> p kt m", p=128))
            nc.sync.dma_start(out=wv_t[:], in_=wv[:, :].rearrange("(kt p) m -> p kt m", p=128))
            bqkv_t = constp.tile([DC, 3], FP32, tag="bqkv")
            ident_t = constp.tile([128, 128], BF16, tag="ident")
            ones_t = constp.tile([1, DH], FP32, tag="ones")
            nc.vector.memset(ones_t[:], 1.0)

            q_b = []
            k_b = []
            v_b = []
            for b in range(B):
                q_b.append(qkvp.tile([128, S], BF16, tag=f"q{b}", name=f"q{b}"))
                k_b.append(qkvp.tile([128, S], BF16, tag=f"k{b}", name=f"k{b}"))
                v_b.append(
                    qkvp.tile([128, JTB, HPC, DH + 1], BF16, tag=f"v{b}", name=f"v{b}")
                )

            late_consts = [False]

            def _proj_one(b, tb, xsrc, xtag, w_t, bcol, dst, warmup=False):
                s0 = tb * TB
                x_t = xinp.tile([128, KT, TB], BF16, tag=xtag, name=xtag, bufs=2)
                nc.gpsimd.dma_start(out=x_t[:], in_=xsrc[b * TPB + tb, :, :, :])
                if warmup and not late_consts[0]:
                    # issued after the first x load so they don't delay it
                    late_consts[0] = True
                    nc.sync.dma_start(out=bqkv_t[:], in_=bqkv[:, :])
                    nc.sync.dma_start(out=ident_t[:], in_=ident[:, :])
                    # dense dummy-matmul burst chained on the first x tile:
                    # identical-weights matmuls stream back-to-back, which is
                    # what fires the PE HAM monitor to K=8/8 (2.4 GHz) before
                    # the projection chain starts, instead of running it cold.
                    warm = psp.tile([128, IC], FP32, tag="ps", name="warm")
                    for _ in range(32):
                        nc.tensor.matmul(
                            warm[:, :128], ident_t[:, :], x_t[:, 0, 0:128],
                            start=True, stop=True,
                        )
                ps_x = psp.tile([128, IC], FP32, tag="ps", name="ps_x")
                for kt in range(KT):
                    nc.tensor.matmul(
                        ps_x[:, :TB], w_t[:, kt, :], x_t[:, kt, :],
                        start=(kt == 0), stop=(kt == KT - 1),
                    )
                nc.vector.tensor_add(
                    dst[:, s0 : s0 + TB],
                    ps_x[:, :TB],
                    bqkv_t[:, bcol : bcol + 1].to_broadcast((DC, TB)),
                )

            def emit_proj_qk(b, tb):
                _proj_one(b, tb, xq, "xq", wq_t, 0, q_b[b], warmup=True)
                _proj_one(b, tb, xk, "xk", wk_t, 1, k_b[b])

            def emit_proj_q(b, tb):
                _proj_one(b, tb, xq, "xq", wq_t, 0, q_b[b])

            def emit_proj_k(b, tb):
                _proj_one(b, tb, xk, "xk", wk_t, 1, k_b[b])

            def emit_proj_v(b, tb):
                # v: project to v^T like q/k (weight-stationary, wide N),
                # add bias, then PE-transpose 128x128 blocks to token-major.
                xv_t = xinp.tile([128, KT, TB], BF16, tag="xv", name="xv_t", bufs=2)
                nc.gpsimd.dma_start(out=xv_t[:], in_=xv[b * TPB + tb, :, :, :])
                ps_v = psp.tile([128, IC], FP32, tag="ps", name="ps_v")
                for kt in range(KT):
                    nc.tensor.matmul(
                        ps_v[:, :TB], wv_t[:, kt, :], xv_t[:, kt, :],
                        start=(kt == 0), stop=(kt == KT - 1),
                    )
                vT = vstgp.tile([128, TB], BF16, tag="vT", name="vT")
                nc.vector.tensor_add(
                    vT[:, :], ps_v[:, :TB], bqkv_t[:, 2:3].to_broadcast((DC, TB))
                )
                for sub in range(VSUB):
                    pt = psp.tile([128, 128], BF16, tag="ps", name="pt")
                    nc.tensor.transpose(
                        pt[:, :], vT[:, sub * 128 : (sub + 1) * 128], ident_t[:, :]
                    )
                    vt_idx = tb * VSUB + sub
                    nc.vector.tensor_copy(
                        v_b[b][:, vt_idx, :, 0:DH],
                        pt[:, :].rearrange("p (h d) -> p h d", h=HPC),
                    )

            pending_final = []

            def emit_attention(b, inject=None):
                for ic in range(NIC):
                    at = [
                        attnp.tile([128, JT, IC], BF16, tag="attn", name=f"at{hh}")
                        for hh in range(HPC)
                    ]
                    pv_ps = [
                        pvp.tile([128, IC], FP32, tag="pv", name=f"pv{hh}")
                        for hh in range(HPC)
                    ]
                    for jt in range(JT):
                        if inject is not None:
                            # must run before this jt's consumers are emitted:
                            # program order defines the data each read sees
                            inject(ic, jt)
                        if jt == 1 and pending_final:
                            # previous chunk's normalization tail, emitted after
                            # this chunk's first scores so the in-order PE
                            # doesn't stall on the reciprocal DMA chain
                            for fn in pending_final:
                                fn()
                            pending_final.clear()
                        s_ps = [
                            psp.tile([128, IC], FP32, tag="ps", name=f"s{hh}")
                            for hh in range(HPC)
                        ]
                        # same-weights matmuls adjacent (they stream at N cycles
                        # with the LDW overlapped); the two heads still overlap
                        # via distinct row groups (partition bases 0/64)
                        for hh in range(HPC):
                            p0 = hh * DH
                            for n in range(NI5):
                                i0 = ic * IC + n * MMW
                                nc.tensor.matmul(
                                    s_ps[hh][:, n * MMW : (n + 1) * MMW],
                                    k_b[b][p0 : p0 + DH, jt * 128 : (jt + 1) * 128],
                                    q_b[b][p0 : p0 + DH, i0 : i0 + MMW],
                                    start=True, stop=True,
                                )
                        for hh in range(HPC):
                            nc.scalar.activation(
                                out=at[hh][:, jt, :], in_=s_ps[hh][:, :],
                                func=mybir.ActivationFunctionType.Exp,
                                scale=SCALE,
                            )
                        # pv accumulation trails the exps by one j-tile
                        for hh in range(HPC):
                            for n in range(NI5):
                                nc.tensor.matmul(
                                    pv_ps[hh][0 : DH + 1, n * MMW : (n + 1) * MMW],
                                    v_b[b][:, jt, hh, :],
                                    at[hh][:, jt, n * MMW : (n + 1) * MMW],
                                    start=(jt == 0), stop=(jt == JT - 1),
                                )
                    for hh in range(HPC):
                        # Evacuate the full [65, IC] pv block to SBUF in one DVE
                        # copy so the psum slot frees immediately (next chunk's
                        # pv accumulation can start); the normalization chain
                        # then runs from SBUF off the critical path.
                        pvsb = outp.tile([DH + 1, IC], FP32, tag="pvsb", name="pvsb")
                        nc.vector.tensor_copy(pvsb[:, :], pv_ps[hh][0 : DH + 1, :])
                        # out = pv[0:64] / pv[64]: reshape the denominator row to
                        # [128, IC/128] for a fast reciprocal, broadcast it back
                        # over 64 partitions with a DMA, multiply on DVE.
                        rsh = drainp.tile([128, IC // 128], FP32, tag="rsh", name="rsh")
                        nc.sync.dma_start(out=rsh[:], in_=pvsb[DH : DH + 1, :])
                        rec = drainp.tile([128, IC // 128], FP32, tag="rec", name="rec")
                        nc.vector.reciprocal(rec[:], rsh[:])
                        rrow = drainp.tile([1, IC], FP32, tag="rrow", name="rrow")
                        nc.sync.dma_start(out=rrow[:], in_=rec[:])

                        def finalize(b=b, ic=ic, hh=hh, pv=pv_ps[hh], pvsb=pvsb, rrow=rrow):
                            # broadcast 1/den over 64 partitions with a K=1
                            # ones-matmul (a replicated-source DMA measures
                            # ~10us; this is ~1us of PE), written into the
                            # unused upper partition rows of the pv psum tile
                            for n in range(NI5):
                                nc.tensor.matmul(
                                    pv[DH : 2 * DH, n * MMW : (n + 1) * MMW],
                                    ones_t[0:1, :],
                                    rrow[0:1, n * MMW : (n + 1) * MMW],
                                    start=True, stop=True,
                                )
                            nc.vector.tensor_mul(
                                pvsb[0:DH, :], pvsb[0:DH, :], pv[DH : 2 * DH, :]
                            )
                            nc.sync.dma_start(
                                out=out[
                                    hh * DH : (hh + 1) * DH,
                                    b * S + ic * IC : b * S + (ic + 1) * IC,
                                ],
                                in_=pvsb[0:DH, :],
                            )

                        pending_final.append(finalize)

            # Prologue: just enough projection for batch-0 attention to start
            # (scores for chunk 0 need q columns 0:IC and the k tiles as the
            # j-loop reaches them). Everything else — remaining b0 projection
            # units and all of b1's — is injected into the attention emission
            # at j-tile milestones, so the kernel is one continuous pipeline
            # and the PE never sits in a long ACT-idle projection phase.
            nc.vector.memset(v_b[0][:, :, :, DH : DH + 1], 1.0)
            emit_proj_qk(0, 0)
            if TPB > 1:
                emit_proj_qk(0, 1)

            # Deadline-aware unit schedule, one small unit per fire point so no
            # injection holds a scores psum slot for long. Deadlines (in jt
            # points of the hosting attention): v(tb) before PV jt=4*tb;
            # k(tb) before scores jt=4*tb; q(tb) before chunk ic reads its
            # columns (point 16 for tb>=2).
            units0 = [(0, lambda: emit_proj_v(0, 0))]
            if TPB > 2:
                units0 += [
                    (3, lambda: emit_proj_v(0, 1)),
                    (5, lambda: emit_proj_k(0, 2)),
                    (7, lambda: emit_proj_v(0, 2)),
                    (9, lambda: emit_proj_k(0, 3)),
                    (11, lambda: emit_proj_v(0, 3)),
                    (13, lambda: emit_proj_q(0, 2)),
                    (15, lambda: emit_proj_q(0, 3)),
                ]
            units1 = []
            if B > 1:
                # b1 q/k spread over the rest of b0's attention; b1 v units go
                # into b1's own attention (needed only by its PV matmuls)
                pts = [16, 18, 20, 22, 24, 26, 28, 30]
                k = 0
                units0.append((16, lambda: nc.vector.memset(v_b[1][:, :, :, DH : DH + 1], 1.0)))
                for tb in range(TPB):
                    units0.append((pts[k % 8], lambda tb_=tb: emit_proj_q(1, tb_))); k += 1
                    units0.append((pts[k % 8], lambda tb_=tb: emit_proj_k(1, tb_))); k += 1
                vdl = [0, 3, 7, 11]
                for tb in range(TPB):
                    units1.append((vdl[tb % 4], lambda tb_=tb: emit_proj_v(1, tb_)))

            def make_inject(units):
                units = sorted(units, key=lambda u: u[0])
                ui = [0]

                def inject(ic, jt):
                    point = ic * JT + jt
                    while ui[0] < len(units) and units[ui[0]][0] <= point:
                        units[ui[0]][1]()
                        ui[0] += 1

                def flush():
                    while ui[0] < len(units):
                        units[ui[0]][1]()
                        ui[0] += 1

                return inject, flush

            inj0, flush0 = make_inject(units0)
            emit_attention(0, inject=inj0)
            flush0()
            if B > 1:
                inj1, flush1 = make_inject(units1)
                emit_attention(1, inject=inj1)
                flush1()
            for fn in pending_final:
                fn()
            pending_final.clear()

    nc.finalize()
    return nc


_PROGRAM_CACHE = {}


def _get_program(S, B):
    key = (S, B)
    if key not in _PROGRAM_CACHE:
        _PROGRAM_CACHE[key] = build_program(S, B)
    return _PROGRAM_CACHE[key]


def make_in_maps(query, key, value, Wq, bq, Wk, bk, Wv, bv):
    S, B, D_ = query.shape
    assert D_ == D
    T = S * B

    TB = 512 if S % 512 == 0 else S
    NTILE = T // TB

    def xt(a):
        # [S, B, D] -> transposed [D, B*S] -> pre-tiled [NTILE, 128, KT, TB]
        # bf16 so each SBUF tile is one contiguous 1MB DMA read.
        aT = np.asarray(a, np.float32).transpose(2, 1, 0).reshape(D_, T)
        a4 = aT.reshape(KT, 128, NTILE, TB).transpose(2, 1, 0, 3)
        return np.ascontiguousarray(a4).astype(NP_BF16)

    xqh, xkh, xvh = xt(query), xt(key), xt(value)
    identh = np.eye(128, dtype=NP_BF16)
    in_maps = []
    for c in range(NCORES):
        rows = slice(c * DC, (c + 1) * DC)
        in_maps.append(
            {
                "xq": xqh, "xk": xkh, "xv": xvh,
                "wq": np.ascontiguousarray(np.asarray(Wq)[rows, :].T).astype(NP_BF16),
                "wk": np.ascontiguousarray(np.asarray(Wk)[rows, :].T).astype(NP_BF16),
                "wv": np.ascontiguousarray(np.asarray(Wv)[rows, :].T).astype(NP_BF16),
                "bqkv": np.ascontiguousarray(
                    np.stack(
                        [np.asarray(bq)[rows], np.asarray(bk)[rows], np.asarray(bv)[rows]],
                        axis=1,
                    )
                ).astype(np.float32),
                "ident": identh,
            }
        )
    return in_maps


def gather_output(results, S, B):
    full = np.empty((S, B, D), np.float32)
    for c in range(NCORES):
        o = np.asarray(results[c]["out"], np.float32)  # [DC, B*S]
        full[:, :, c * DC : (c + 1) * DC] = o.reshape(DC, B, S).transpose(2, 1, 0)
    return full


def kernel(query, key, value, Wq, bq, Wk, bk, Wv, bv):
    from concourse.bass_utils import run_bass_kernel_spmd

    S, B, _ = query.shape
    nc = _get_program(S, B)
    in_maps = make_in_maps(query, key, value, Wq, bq, Wk, bk, Wv, bv)
    res = run_bass_kernel_spmd(nc, in_maps, list(range(NCORES)))
    return gather_output(res.results, S, B)

